# revision 1
# baseline (speedup 1.0000x reference)
"""EqualizedModulatedConv2d (StyleGAN2) Trainium2 kernel.

Strategy: data-parallel over batch B=16 across 8 NeuronCores (2 samples/core).
Each core runs the full pipeline for its samples:
  1. style FC: esT[i,b] = elr * (lin_scale * (style @ fcW.T)[b,i] + fc_bias[i])
  2. w2T[i,o] = sum_t wT[i,o,t]^2 (from f32r-rounded weights)
  3. denomT[o,b] = sum_i w2T[i,o] * esT[i,b]^2 ; normT = 1/sqrt(denom + 1e-8)
  4. xm = x * esT (per in-channel, per sample) -> rounded to f32r
  5. conv: implicit GEMM, 9 taps x 4 iC chunks accumulated in PSUM (f32r
     matmuls, free dim 512 = 8 rows x 64 cols of the 66-wide padded image)
  6. demod: out = acc * normT during PSUM->SBUF copy, then DMA out.

Host side: pads x spatially (66x66), transposes weight to [iC, oC, 9],
fc_weight to [S, iC], style to [S, B]; gathers per-core outputs.
"""
import numpy as np

B, IC, OC, K, H, W, S = 16, 512, 512, 3, 64, 64, 512
NCORES = 8
BL = B // NCORES          # samples per core
PW = W + 2                # padded width
RT = 8                    # output rows per tile
NRT = H // RT             # row tiles
ICC = IC // 128           # in-channel chunks
OCC = OC // 128           # out-channel chunks
SC = S // 128             # style-dim chunks
ELR = (2.0 / (IC * K * K)) ** 0.5
LIN = (2.0 / S) ** 0.5

_CACHE = {}


def _build():
    import concourse.bacc as bacc
    import concourse.mybir as mybir
    import concourse.tile as tile

    f32 = mybir.dt.float32
    f32r = mybir.dt.float32r
    ALU = mybir.AluOpType

    nc = bacc.Bacc(None, target_bir_lowering=False, debug=False)
    xp = nc.dram_tensor("xp", [BL, IC, H + 2, PW], f32, kind="ExternalInput").ap()
    wt = nc.dram_tensor("wt", [IC, OC, K * K], f32, kind="ExternalInput").ap()
    fcw = nc.dram_tensor("fcw", [S, IC], f32, kind="ExternalInput").ap()
    st = nc.dram_tensor("st", [S, BL], f32, kind="ExternalInput").ap()
    fcb = nc.dram_tensor("fcb", [IC, 1], f32, kind="ExternalInput").ap()
    y = nc.dram_tensor("y", [BL, OC, H, W], f32, kind="ExternalOutput").ap()

    TX = W // 2          # 32 winograd tiles along x
    NR = 4               # winograd taps

    with tile.TileContext(nc) as tc:
        with (
            tc.tile_pool(name="up", bufs=1) as up,
            tc.tile_pool(name="wsp", bufs=3) as wsp,
            tc.tile_pool(name="fcp", bufs=1) as fcp,
            tc.tile_pool(name="sml", bufs=1) as sml,
            tc.tile_pool(name="w2t", bufs=1) as w2t,
            tc.tile_pool(name="xin", bufs=2) as xinp,
            tc.tile_pool(name="xmp", bufs=2) as xmp,
            tc.tile_pool(name="vp", bufs=8) as vp,
            tc.tile_pool(name="itp", bufs=3) as itp,
            tc.tile_pool(name="outp", bufs=2) as outp,
            tc.tile_pool(name="acc", bufs=6, space="PSUM") as accp,
            tc.tile_pool(name="pacc", bufs=2, space="PSUM") as paccp,
        ):
            # ---- fc params ----
            st_sb = fcp.tile([128, SC, BL], f32)
            nc.sync.dma_start(st_sb[:], st.rearrange("(sc p) b -> p sc b", p=128))
            fcb_sb = fcp.tile([128, ICC], f32)
            nc.sync.dma_start(fcb_sb[:], fcb.rearrange("(ic p) z -> p (ic z)", p=128))
            fcw_r = fcw.rearrange("(sc p) i -> p sc i", p=128)
            fcw_sbs = []
            for sc in range(SC):
                fcw_chunk = fcp.tile([128, IC], f32, tag=f"fcw{sc}")
                nc.scalar.dma_start(fcw_chunk[:], fcw_r[:, sc, :])
                fcw_sbs.append(fcw_chunk)

            # ---- style FC -> esT[i, b] = elr*s ----
            ebias = sml.tile([128, ICC], f32)
            nc.scalar.mul(ebias[:], fcb_sb[:], ELR)
            es_sbs, ss_sbs = [], []
            for ic in range(ICC):
                ps = paccp.tile([128, BL], f32, tag="pp")
                for sc in range(SC):
                    nc.tensor.matmul(
                        ps[:], fcw_sbs[sc][:, ic * 128:(ic + 1) * 128], st_sb[:, sc, :],
                        start=(sc == 0), stop=(sc == SC - 1),
                    )
                es_c = sml.tile([128, BL], f32, tag=f"es{ic}")
                nc.scalar.activation(
                    es_c[:], ps[:], mybir.ActivationFunctionType.Identity,
                    bias=ebias[:, ic:ic + 1], scale=ELR * LIN,
                )
                ss_c = sml.tile([128, BL], f32, tag=f"ss{ic}")
                nc.vector.tensor_mul(ss_c[:], es_c[:], es_c[:])
                es_sbs.append(es_c)
                ss_sbs.append(ss_c)

            # ---- x load + modulate + winograd input transform ----
            xp_r = xp.rearrange("b (ic p) r c -> b ic p (r c)", p=128)
            xm_cache = {}

            def load_v(b, rt):
                if (b, rt) in xm_cache:
                    return xm_cache.pop((b, rt))
                r0 = rt * RT
                vs = []
                for ic in range(ICC):
                    xin = xinp.tile([128, (RT + 2) * PW], f32, tag="xin")
                    nc.sync.dma_start(
                        xin[:], xp_r[b, ic, :, r0 * PW:(r0 + RT + 2) * PW]
                    )
                    xmt = xmp.tile([128, (RT + 2) * PW], f32, tag="xm")
                    nc.scalar.mul(xmt[:], xin[:], es_sbs[ic][:, b:b + 1])
                    xv = xmt.rearrange("p (r two k) -> p r two k", two=2, k=PW // 2)
                    d0 = xv[:, :, 0, 0:TX]
                    d1 = xv[:, :, 1, 0:TX]
                    d2 = xv[:, :, 0, 1:TX + 1]
                    d3 = xv[:, :, 1, 1:TX + 1]
                    vt = vp.tile([128, NR, RT + 2, TX], f32r, tag="v")
                    nc.vector.tensor_sub(vt[:, 0], d0, d2)
                    nc.vector.tensor_add(vt[:, 1], d1, d2)
                    nc.vector.tensor_sub(vt[:, 2], d2, d1)
                    nc.vector.tensor_sub(vt[:, 3], d1, d3)
                    vs.append(vt)
                return vs

            # ---- weights: stream chunks, build winograd taps u + w2 ----
            wt_r = wt.rearrange("(ic p) o t -> p ic o t", p=128)
            u_sbs = []
            for ic in range(ICC):
                u_chunk = up.tile([128, OC, K, NR], f32r, tag=f"u{ic}")
                u_sbs.append(u_chunk)
            w2_sbs = {}
            for ic in range(ICC):
                for oc in range(OCC):
                    w2s = sml.tile([128, 128], f32, tag=f"w2_{ic}_{oc}")
                    w2_sbs[(ic, oc)] = w2s

            def load_wt(ic, oc):
                sl = slice(oc * 128, (oc + 1) * 128)
                ws = wsp.tile([128, 128, K, K], f32, tag="ws")
                nc.sync.dma_start(
                    ws.rearrange("p o a b -> p (o a b)"),
                    wt_r[:, ic, sl, :].rearrange("p o t -> p (o t)"),
                )
                # w2 slice for demod norm
                sq = w2t.tile([128, 128, K * K], f32, tag="w2tmp")
                wv = ws.rearrange("p o a b -> p o (a b)")
                nc.scalar.square(sq[:], wv)
                nc.vector.reduce_sum(w2_sbs[(ic, oc)][:], sq[:],
                                     axis=mybir.AxisListType.X)
                # winograd taps: u0=w0, u1=(w0+w1+w2)/2, u2=(w0-w1+w2)/2, u3=w2
                u = u_sbs[ic]
                w0, w1, w2_ = ws[:, :, :, 0], ws[:, :, :, 1], ws[:, :, :, 2]
                nc.gpsimd.tensor_copy(u[:, sl, :, 0], w0)
                nc.gpsimd.tensor_copy(u[:, sl, :, 3], w2_)
                s02 = w2t.tile([128, 128, K], f32, tag="s02")
                nc.gpsimd.tensor_add(s02[:], w0, w2_)
                w1h = w2t.tile([128, 128, K], f32, tag="w1h")
                nc.scalar.mul(w1h[:], w1, 0.5)
                nc.vector.scalar_tensor_tensor(
                    u[:, sl, :, 1], s02[:], 0.5, w1h[:], ALU.mult, ALU.add)
                nc.vector.scalar_tensor_tensor(
                    u[:, sl, :, 2], s02[:], 0.5, w1h[:], ALU.mult, ALU.subtract)

            load_wt(0, 0)
            xm_cache[(0, 0)] = load_v(0, 0)
            for ic in range(1, ICC):
                load_wt(ic, 0)
            xm_cache[(0, 1)] = load_v(0, 1)
            for oc in range(1, OCC):
                for ic in range(ICC):
                    load_wt(ic, oc)

            # ---- demod norm: normT[o, b] (per-oc as w2 slices land) ----
            norm_sb = sml.tile([128, OCC, BL], f32)
            sqd = sml.tile([128, OCC, BL], f32)
            eps_sb = sml.tile([128, 1], f32)
            nc.vector.memset(eps_sb[:], 1e-8)
            for oc in range(OCC):
                pd = paccp.tile([128, BL], f32, tag="pp")
                for ic in range(ICC):
                    nc.tensor.matmul(
                        pd[:], w2_sbs[(ic, oc)][:], ss_sbs[ic][:],
                        start=(ic == 0), stop=(ic == ICC - 1),
                    )
                nc.scalar.activation(
                    sqd[:, oc, :], pd[:], mybir.ActivationFunctionType.Sqrt,
                    bias=eps_sb[:],
                )
                nc.vector.reciprocal(norm_sb[:, oc, :], sqd[:, oc, :])

            # ---- main winograd-conv loop ----
            def conv_group(b, rt, vs, oc):
                    r0 = rt * RT
                    if True:
                        osl = slice(oc * 128, (oc + 1) * 128)
                        psA = accp.tile([128, 2, RT * TX], f32, tag="wacc")
                        psB = accp.tile([128, 2, RT * TX], f32, tag="wacc")
                        for r in range(NR):
                            ps = psA if r < 2 else psB
                            j = r % 2
                            for ic in range(ICC):
                                for dy in range(K):
                                    nc.tensor.matmul(
                                        ps[:, j, :],
                                        u_sbs[ic][:, osl, dy, r],
                                        vs[ic][:, r, dy:dy + RT, :],
                                        start=(ic == 0 and dy == 0),
                                        stop=(ic == ICC - 1 and dy == K - 1),
                                    )
                        # inverse transform + demod + store
                        m0, m1 = psA[:, 0, :], psA[:, 1, :]
                        m2, m3 = psB[:, 0, :], psB[:, 1, :]
                        nv = norm_sb[:, oc, b:b + 1]
                        c1 = itp.tile([128, RT * TX], f32, tag="it")
                        nc.scalar.copy(c1[:], m1)
                        a01 = itp.tile([128, RT * TX], f32, tag="it")
                        nc.vector.tensor_add(a01[:], c1[:], m0)
                        t012 = itp.tile([128, RT * TX], f32, tag="it")
                        nc.vector.tensor_add(t012[:], a01[:], m2)
                        b13 = itp.tile([128, RT * TX], f32, tag="it")
                        nc.vector.tensor_sub(b13[:], c1[:], m3)
                        t123 = itp.tile([128, RT * TX], f32, tag="it")
                        nc.vector.tensor_sub(t123[:], b13[:], m2)
                        ot = outp.tile([128, RT * W], f32, tag="ot")
                        ov = ot.rearrange("p (r k two) -> p r k two", two=2, k=TX)
                        tv0 = t012.rearrange("p (r k) -> p r k", k=TX)
                        tv1 = t123.rearrange("p (r k) -> p r k", k=TX)
                        nc.scalar.mul(ov[:, :, :, 0], tv0, nv)
                        nc.scalar.mul(ov[:, :, :, 1], tv1, nv)
                        nc.sync.dma_start(
                            y[b, osl, r0:r0 + RT, :].rearrange("p r c -> p (r c)"),
                            ot[:],
                        )

            # first two row-tiles of b0 interleaved oc-outer: each arriving
            # weight column-chunk enables 2 groups of PE work during the
            # initial weight stream
            vs00 = load_v(0, 0)
            vs01 = load_v(0, 1)
            for oc in range(2):
                conv_group(0, 0, vs00, oc)
                conv_group(0, 1, vs01, oc)
            conv_group(0, 0, vs00, 2)
            conv_group(0, 0, vs00, 3)
            conv_group(0, 1, vs01, 2)
            conv_group(0, 1, vs01, 3)
            for b in range(BL):
                for rt in range(NRT):
                    if b == 0 and rt < 2:
                        continue
                    vs = load_v(b, rt)
                    for oc in range(OCC):
                        conv_group(b, rt, vs, oc)
    nc.compile()
    return nc


class _Runner:
    """Persistent jitted PJRT executor for the SPMD kernel (axon path)."""

    def __init__(self, nc, n_cores):
        import jax
        import numpy as np
        from jax.sharding import Mesh, PartitionSpec
        try:
            from jax.experimental.shard_map import shard_map
        except ImportError:
            from jax.shard_map import shard_map
        import concourse.mybir as mybir
        from concourse.bass2jax import (
            _bass_exec_p, install_neuronx_cc_hook, partition_id_tensor,
        )

        install_neuronx_cc_hook()
        self.jax = jax
        self.n_cores = n_cores
        partition_name = (
            nc.partition_id_tensor.name if nc.partition_id_tensor else None
        )
        in_names, out_names, out_avals, zero_outs = [], [], [], []
        for alloc in nc.m.functions[0].allocations:
            if not isinstance(alloc, mybir.MemoryLocationSet):
                continue
            name = alloc.memorylocations[0].name
            if alloc.kind == "ExternalInput":
                if name != partition_name:
                    in_names.append(name)
            elif alloc.kind == "ExternalOutput":
                out_names.append(name)
                shape = tuple(alloc.tensor_shape)
                dtype = mybir.dt.np(alloc.dtype)
                out_avals.append(jax.core.ShapedArray(shape, dtype))
                zero_outs.append(np.zeros(shape, dtype))
        self.in_names, self.out_names, self.out_avals = in_names, out_names, out_avals

        def _body(*args):
            operands = list(args)
            if partition_name is not None:
                operands.append(partition_id_tensor())
            return tuple(
                _bass_exec_p.bind(
                    *operands,
                    out_avals=tuple(out_avals),
                    in_names=tuple(in_names + out_names + ([partition_name] if partition_name else [])),
                    out_names=tuple(out_names),
                    lowering_input_output_aliases=(),
                    sim_require_finite=False,
                    sim_require_nnan=False,
                    nc=nc,
                )
            )

        devices = jax.devices()[:n_cores]
        mesh = Mesh(np.asarray(devices), ("core",))
        n_params = len(in_names)
        self.fn = jax.jit(
            shard_map(
                _body, mesh=mesh,
                in_specs=(PartitionSpec("core"),) * (n_params + len(out_names)),
                out_specs=(PartitionSpec("core"),) * len(out_names),
                check_rep=False,
            ),
            keep_unused=True,
        )
        self.sharding = jax.sharding.NamedSharding(mesh, PartitionSpec("core"))
        self._dev_zeros = [
            jax.device_put(
                np.zeros((n_cores * z.shape[0], *z.shape[1:]), z.dtype), self.sharding
            )
            for z in zero_outs
        ]

    def put_inputs(self, in_maps):
        concat = [
            np.concatenate(
                [np.asarray(in_maps[c][n]) for c in range(self.n_cores)], axis=0
            )
            for n in self.in_names
        ]
        return [self.jax.device_put(a, self.sharding) for a in concat]

    def run(self, dev_args):
        outs = self.fn(*dev_args, *self._dev_zeros)
        self.jax.block_until_ready(outs)
        return outs

    def results(self, outs):
        res = []
        for c in range(self.n_cores):
            d = {}
            for i, name in enumerate(self.out_names):
                full = np.asarray(outs[i])
                d[name] = full.reshape(self.n_cores, *self.out_avals[i].shape)[c]
            res.append(d)
        return res


def _get_runner():
    if "runner" not in _CACHE:
        nc = _build()
        _CACHE["nc"] = nc
        _CACHE["runner"] = _Runner(nc, NCORES)
    return _CACHE["runner"]


def _prep_inputs(x, style, weight, fc_weight, fc_bias):
    """Host-side sharding + layout marshalling. Returns per-core input maps."""
    x = np.asarray(x, dtype=np.float32)
    style = np.asarray(style, dtype=np.float32)
    weight = np.asarray(weight, dtype=np.float32)
    fc_weight = np.asarray(fc_weight, dtype=np.float32)
    fc_bias = np.asarray(fc_bias, dtype=np.float32)

    xpad = np.zeros((B, IC, H + 2, PW), dtype=np.float32)
    xpad[:, :, 1:H + 1, 1:W + 1] = x
    # de-interleave columns: row layout [even cols | odd cols] so the
    # winograd input-transform reads contiguous runs
    xpad = np.ascontiguousarray(
        xpad.reshape(B, IC, H + 2, PW // 2, 2).transpose(0, 1, 2, 4, 3)
    ).reshape(B, IC, H + 2, PW)
    wt_host = np.ascontiguousarray(
        weight.transpose(1, 0, 2, 3).reshape(IC, OC, K * K)
    )
    fcw_host = np.ascontiguousarray(fc_weight.T)
    fcb_host = np.ascontiguousarray(fc_bias.reshape(IC, 1))

    in_maps = []
    for c in range(NCORES):
        sl = slice(c * BL, (c + 1) * BL)
        in_maps.append({
            "xp": np.ascontiguousarray(xpad[sl]),
            "wt": wt_host,
            "fcw": fcw_host,
            "st": np.ascontiguousarray(style[sl].T),
            "fcb": fcb_host,
        })
    return in_maps


def kernel(x, style, weight, fc_weight, fc_bias):
    runner = _get_runner()
    in_maps = _prep_inputs(x, style, weight, fc_weight, fc_bias)
    dev_args = runner.put_inputs(in_maps)
    outs = runner.run(dev_args)
    res = runner.results(outs)
    out = np.concatenate([res[c]["y"] for c in range(NCORES)], axis=0)
    return out.astype(np.float32)



# revision 6
# speedup vs baseline: 1.1925x; 1.1925x over previous
"""EqualizedModulatedConv2d (StyleGAN2) Trainium2 kernel.

Strategy: data-parallel over batch B=16 across 8 NeuronCores (2 samples/core).
Conv algorithm: F(4,3) Winograd along the x-dim (6 taps -> 4 output cols),
direct accumulation along y (3 dy taps folded into the PSUM accumulation),
fp16 matmul operands with fp32 PSUM accumulate.

Per core:
  1. style FC (PE, f32) -> es[i,b] = elr*(lin*(style @ fcW.T)[b,i] + fc_bias)
  2. demod norm from host-precomputed w2[i,o]: denom = elr^2 * (w2.T @ es^2),
     norm = rsqrt(denom + 1e-8)   (PE + Act + DVE, tiny)
  3. modulate: xm = x * es (Act engine, fp16), x shipped fp16,
     column-phase-deinterleaved (4 phases x 17) so Winograd input-transform
     reads are stride-1 (enables the DVE 4x perf mode)
  4. input transform: 12 scalar_tensor_tensor ops per (sample, half, icChunk)
     building V[6 taps][35 rows][16 x-tiles] fp16
  5. conv: per (sample, half, rowTile16, ocChunk): 6 taps x 3 dy x 4 ic = 72
     fp16 matmuls, free dim 256 (16 rows x 16 tiles), accumulating 6 tap
     planes in PSUM
  6. inverse transform A^T m: pair-sums on Pool engine, combines on DVE
     (fp16, 4x mode), demod scale + col re-interleave on Act, DMA out.

Host side: winograd weight transform U = G @ w (f64->fp16), w2 = sum(w^2),
x padding + phase deinterleave + fp16 cast, per-core batch sharding.
"""
import numpy as np

B, IC, OC, K, H, W, S = 16, 512, 512, 3, 64, 64, 512
NCORES = 8
BL = B // NCORES          # samples per core
ICC = IC // 128
OCC = OC // 128
SC = S // 128
NT = 6                    # winograd taps F(4,3)
XT = W // 4               # 16 x-tiles per row
NPH = 17                  # phase width (68 padded cols / 4 phases)
PW = 4 * NPH              # 68 padded width
HR = 35                   # rows per half (padded rows 0..34 / 31..65)
RT = 16                   # output rows per conv group
ELR = (2.0 / (IC * K * K)) ** 0.5
LIN = (2.0 / S) ** 0.5

_CACHE = {}

# F(4,3) winograd weight transform (host side, f64)
_G = np.array([
    [1 / 4, 0, 0],
    [-1 / 6, -1 / 6, -1 / 6],
    [-1 / 6, 1 / 6, -1 / 6],
    [1 / 24, 1 / 12, 1 / 6],
    [1 / 24, -1 / 12, 1 / 6],
    [0, 0, 1],
], dtype=np.float64)


def _build():
    import concourse.bacc as bacc
    import concourse.mybir as mybir
    import concourse.tile as tile

    f32 = mybir.dt.float32
    f16 = mybir.dt.float16
    ALU = mybir.AluOpType
    AF = mybir.ActivationFunctionType

    nc = bacc.Bacc(None, target_bir_lowering=False, debug=False)
    xph = nc.dram_tensor("xph", [BL, ICC, 2, 128, HR * PW], f16,
                         kind="ExternalInput").ap()
    ut = nc.dram_tensor("ut", [ICC, 128, OCC, 128 * K * NT], f16,
                        kind="ExternalInput").ap()
    w2d = nc.dram_tensor("w2d", [IC, OC], f32, kind="ExternalInput").ap()
    fcw = nc.dram_tensor("fcw", [S, IC], f32, kind="ExternalInput").ap()
    st = nc.dram_tensor("st", [S, BL], f32, kind="ExternalInput").ap()
    fcb = nc.dram_tensor("fcb", [IC, 1], f32, kind="ExternalInput").ap()
    y = nc.dram_tensor("y", [BL, OC, H, W], f32, kind="ExternalOutput").ap()

    with tile.TileContext(nc) as tc:
        with (
            tc.tile_pool(name="up", bufs=1) as up,
            tc.tile_pool(name="fcp", bufs=1) as fcp,
            tc.tile_pool(name="sml", bufs=1) as sml,
            tc.tile_pool(name="xin", bufs=2) as xinp,
            tc.tile_pool(name="xmp", bufs=2) as xmp,
            tc.tile_pool(name="scr", bufs=2) as scrp,
            tc.tile_pool(name="vp", bufs=2) as vp,
            tc.tile_pool(name="ivp", bufs=2) as ivp,
            tc.tile_pool(name="outp", bufs=3) as outp,
            tc.tile_pool(name="acc", bufs=2, space="PSUM") as accp,
            tc.tile_pool(name="pacc", bufs=2, space="PSUM") as paccp,
        ):
            # ---- param DMAs (sync queue) ----
            st_sb = fcp.tile([128, SC, BL], f32)
            nc.sync.dma_start(st_sb[:], st.rearrange("(sc p) b -> p sc b", p=128))
            fcb_sb = fcp.tile([128, ICC], f32)
            nc.sync.dma_start(fcb_sb[:], fcb.rearrange("(ic p) z -> p (ic z)", p=128))
            fcw_sb = fcp.tile([128, SC, IC], f32)
            nc.sync.dma_start(fcw_sb[:], fcw.rearrange("(sc p) i -> p sc i", p=128))
            w2_sb = fcp.tile([128, ICC, OC], f32)
            nc.sync.dma_start(w2_sb[:], w2d.rearrange("(ic p) o -> p ic o", p=128))

            # ---- U taps DMA (gpsimd queue, oc-major so oc0 lands first) ----
            u_sb = up.tile([128, ICC, OCC, 128, K, NT], f16)
            for oc in range(OCC):
                for ic in range(ICC):
                    nc.gpsimd.dma_start(
                        u_sb[:, ic, oc].rearrange("p o a b -> p (o a b)"),
                        ut[ic, :, oc, :],
                    )

            # ---- style FC -> es[i, b] = elr*s ----
            ebias = sml.tile([128, ICC], f32)
            nc.scalar.mul(ebias[:], fcb_sb[:], ELR)
            es_sb = sml.tile([128, ICC, BL], f32)
            ss_sb = sml.tile([128, ICC, BL], f32)
            for ic in range(ICC):
                ps = paccp.tile([128, BL], f32, tag="pp")
                for sc in range(SC):
                    nc.tensor.matmul(
                        ps[:], fcw_sb[:, sc, ic * 128:(ic + 1) * 128],
                        st_sb[:, sc, :],
                        start=(sc == 0), stop=(sc == SC - 1),
                    )
                nc.scalar.activation(
                    es_sb[:, ic, :], ps[:], AF.Identity,
                    bias=ebias[:, ic:ic + 1], scale=ELR * LIN,
                )
            nc.vector.tensor_mul(
                ss_sb.rearrange("p i b -> p (i b)"),
                es_sb.rearrange("p i b -> p (i b)"),
                es_sb.rearrange("p i b -> p (i b)"),
            )

            # ---- demod norm: norm[o, oc, b] ----
            norm_sb = sml.tile([128, OCC, BL], f32)
            sqd = sml.tile([128, OCC, BL], f32)
            eps_sb = sml.tile([128, 1], f32)
            nc.vector.memset(eps_sb[:], 1e-8)
            for oc in range(OCC):
                pd = paccp.tile([128, BL], f32, tag="pp")
                for ic in range(ICC):
                    nc.tensor.matmul(
                        pd[:], w2_sb[:, ic, oc * 128:(oc + 1) * 128],
                        ss_sb[:, ic, :],
                        start=(ic == 0), stop=(ic == ICC - 1),
                    )
                nc.scalar.activation(
                    sqd[:, oc, :], pd[:], AF.Sqrt,
                    bias=eps_sb[:],
                )
                nc.vector.reciprocal(norm_sb[:, oc, :], sqd[:, oc, :])

            # ---- V build: modulate (Act) + input transform (DVE stt) ----
            def build_v(vtile, b, h, ic):
                xin = xinp.tile([128, HR * PW], f16, tag="xin")
                nc.sync.dma_start(xin[:], xph[b, ic, h])
                xm = xmp.tile([128, HR * PW], f16, tag="xm")
                nc.scalar.mul(xm[:], xin[:], es_sb[:, ic, b:b + 1])
                xv = xm.rearrange("p (r ph t) -> p r ph t", ph=4, t=NPH)
                d0 = xv[:, :, 0, 0:XT]
                d1 = xv[:, :, 1, 0:XT]
                d2 = xv[:, :, 2, 0:XT]
                d3 = xv[:, :, 3, 0:XT]
                d4 = xv[:, :, 0, 1:XT + 1]
                d5 = xv[:, :, 1, 1:XT + 1]
                s6 = scrp.tile([128, 6, HR, XT], f16, tag="s6")
                q, p, u, v, z, zz = (s6[:, j] for j in range(6))
                V = vtile[:, ic]
                stt = nc.vector.scalar_tensor_tensor
                stt(q, d2, -4.0, d4, ALU.mult, ALU.add)
                stt(p, d1, -4.0, d3, ALU.mult, ALU.add)
                stt(u, d1, -1.0, d3, ALU.mult, ALU.add)
                stt(v, d2, -1.0, d4, ALU.mult, ALU.add)
                stt(z, d0, 1.0, d2, ALU.mult, ALU.subtract)
                stt(zz, d3, -1.0, d5, ALU.mult, ALU.add)
                stt(V[:, 0], z, 4.0, v, ALU.mult, ALU.add)
                stt(V[:, 1], p, 1.0, q, ALU.mult, ALU.add)
                stt(V[:, 2], p, -1.0, q, ALU.mult, ALU.add)
                stt(V[:, 3], u, 2.0, v, ALU.mult, ALU.add)
                stt(V[:, 4], u, -2.0, v, ALU.mult, ALU.add)
                stt(V[:, 5], u, -4.0, zz, ALU.mult, ALU.add)

            # ---- conv group: 72 matmuls + inverse + demod + store ----
            def conv_group(vtile, b, h, rt, oc):
                ps = accp.tile([128, NT, RT * XT], f32, tag="acc")
                base = 16 * rt + (1 if h else 0)
                osl = slice(oc * 128, (oc + 1) * 128)
                for tap in range(NT):
                    pview = ps[:, tap, :]
                    for dy in range(K):
                        r0 = base + dy
                        for ic in range(ICC):
                            nc.tensor.matmul(
                                pview,
                                u_sb[:, ic, oc, :, dy, tap],
                                vtile[:, ic, tap, r0:r0 + RT, :].rearrange(
                                    "p r t -> p (r t)"),
                                start=(dy == 0 and ic == 0),
                                stop=(dy == K - 1 and ic == ICC - 1),
                            )
                # inverse transform: odd taps m1,m3 -> SBUF via Act, then
                # each DVE op reads at most one PSUM operand
                c13 = ivp.tile([128, 2, RT * XT], f16, tag="c13")
                nc.scalar.copy(c13[:], ps[:, 1:5:2, :])
                PR = ivp.tile([128, 2, RT * XT], f16, tag="PR")
                QS = ivp.tile([128, 2, RT * XT], f16, tag="QS")
                stt = nc.vector.scalar_tensor_tensor
                stt(PR[:], c13[:], 1.0, ps[:, 2:6:2, :], ALU.mult, ALU.add)
                stt(QS[:], c13[:], 1.0, ps[:, 2:6:2, :], ALU.mult, ALU.subtract)
                sc2 = ivp.tile([128, 2, RT * XT], f16, tag="sc2")
                stt(sc2[:, 0], PR[:, 0], 1.0, PR[:, 1], ALU.mult, ALU.add)
                stt(sc2[:, 1], QS[:, 1], 8.0, QS[:, 0], ALU.mult, ALU.add)
                o03 = ivp.tile([128, 2, RT * XT], f16, tag="o03")
                stt(o03[:], ps[:, 0:6:5, :], 1.0, sc2[:], ALU.mult, ALU.add)
                o12 = ivp.tile([128, 2, RT * XT], f16, tag="o12")
                stt(o12[:, 0], QS[:, 1], 2.0, QS[:, 0], ALU.mult, ALU.add)
                stt(o12[:, 1], PR[:, 1], 4.0, PR[:, 0], ALU.mult, ALU.add)
                # demod scale + column re-interleave on Act
                ot = outp.tile([128, RT * W], f32, tag="ot")
                ov = ot.rearrange("p (r t four) -> p r t four", four=4, t=XT)
                nv = norm_sb[:, oc, b:b + 1]
                o03v = o03.rearrange("p two (r t) -> p two r t", t=XT)
                o12v = o12.rearrange("p two (r t) -> p two r t", t=XT)
                nc.scalar.mul(ov[:, :, :, 0], o03v[:, 0], nv)
                nc.scalar.mul(ov[:, :, :, 1], o12v[:, 0], nv)
                nc.scalar.mul(ov[:, :, :, 2], o12v[:, 1], nv)
                nc.scalar.mul(ov[:, :, :, 3], o03v[:, 1], nv)
                r0g = 32 * h + 16 * rt
                nc.sync.dma_start(
                    y[b, osl, r0g:r0g + RT, :].rearrange("p r c -> p (r c)"),
                    ot[:],
                )

            # ---- main pipeline ----
            steps = [(b, h) for b in range(BL) for h in range(2)]
            vtiles = []
            v0 = vp.tile([128, ICC, NT, HR, XT], f16, tag="V")
            for ic in range(ICC):
                build_v(v0, steps[0][0], steps[0][1], ic)
            vtiles.append(v0)
            for i, (b, h) in enumerate(steps):
                vt = vtiles[i]
                if i + 1 < len(steps):
                    vnext = vp.tile([128, ICC, NT, HR, XT], f16, tag="V")
                    vtiles.append(vnext)
                    nb, nh = steps[i + 1]
                    pending = list(range(ICC))
                else:
                    vnext, pending = None, []
                gi = 0
                for rt in range(2):
                    for oc in range(OCC):
                        conv_group(vt, b, h, rt, oc)
                        if gi < len(pending):
                            build_v(vnext, nb, nh, pending[gi])
                        gi += 1
    nc.compile()
    return nc


class _Runner:
    """Persistent jitted PJRT executor for the SPMD kernel (axon path)."""

    def __init__(self, nc, n_cores):
        import jax
        import numpy as np
        from jax.sharding import Mesh, PartitionSpec
        try:
            from jax.experimental.shard_map import shard_map
        except ImportError:
            from jax.shard_map import shard_map
        import concourse.mybir as mybir
        from concourse.bass2jax import (
            _bass_exec_p, install_neuronx_cc_hook, partition_id_tensor,
        )

        install_neuronx_cc_hook()
        self.jax = jax
        self.n_cores = n_cores
        partition_name = (
            nc.partition_id_tensor.name if nc.partition_id_tensor else None
        )
        in_names, out_names, out_avals, zero_outs = [], [], [], []
        for alloc in nc.m.functions[0].allocations:
            if not isinstance(alloc, mybir.MemoryLocationSet):
                continue
            name = alloc.memorylocations[0].name
            if alloc.kind == "ExternalInput":
                if name != partition_name:
                    in_names.append(name)
            elif alloc.kind == "ExternalOutput":
                out_names.append(name)
                shape = tuple(alloc.tensor_shape)
                dtype = mybir.dt.np(alloc.dtype)
                out_avals.append(jax.core.ShapedArray(shape, dtype))
                zero_outs.append(np.zeros(shape, dtype))
        self.in_names, self.out_names, self.out_avals = in_names, out_names, out_avals

        def _body(*args):
            operands = list(args)
            if partition_name is not None:
                operands.append(partition_id_tensor())
            return tuple(
                _bass_exec_p.bind(
                    *operands,
                    out_avals=tuple(out_avals),
                    in_names=tuple(in_names + out_names + ([partition_name] if partition_name else [])),
                    out_names=tuple(out_names),
                    lowering_input_output_aliases=(),
                    sim_require_finite=False,
                    sim_require_nnan=False,
                    nc=nc,
                )
            )

        devices = jax.devices()[:n_cores]
        mesh = Mesh(np.asarray(devices), ("core",))
        n_params = len(in_names)
        self.fn = jax.jit(
            shard_map(
                _body, mesh=mesh,
                in_specs=(PartitionSpec("core"),) * (n_params + len(out_names)),
                out_specs=(PartitionSpec("core"),) * len(out_names),
                check_rep=False,
            ),
            keep_unused=True,
        )
        self.sharding = jax.sharding.NamedSharding(mesh, PartitionSpec("core"))
        self._dev_zeros = [
            jax.device_put(
                np.zeros((n_cores * z.shape[0], *z.shape[1:]), z.dtype), self.sharding
            )
            for z in zero_outs
        ]

    def put_inputs(self, in_maps):
        concat = [
            np.concatenate(
                [np.asarray(in_maps[c][n]) for c in range(self.n_cores)], axis=0
            )
            for n in self.in_names
        ]
        return [self.jax.device_put(a, self.sharding) for a in concat]

    def run(self, dev_args):
        outs = self.fn(*dev_args, *self._dev_zeros)
        self.jax.block_until_ready(outs)
        return outs

    def results(self, outs):
        res = []
        for c in range(self.n_cores):
            d = {}
            for i, name in enumerate(self.out_names):
                full = np.asarray(outs[i])
                d[name] = full.reshape(self.n_cores, *self.out_avals[i].shape)[c]
            res.append(d)
        return res


def _get_runner():
    if "runner" not in _CACHE:
        nc = _build()
        _CACHE["nc"] = nc
        _CACHE["runner"] = _Runner(nc, NCORES)
    return _CACHE["runner"]


def _prep_inputs(x, style, weight, fc_weight, fc_bias):
    """Host-side sharding + layout marshalling. Returns per-core input maps."""
    x = np.asarray(x, dtype=np.float32)
    style = np.asarray(style, dtype=np.float32)
    weight = np.asarray(weight, dtype=np.float32)
    fc_weight = np.asarray(fc_weight, dtype=np.float32)
    fc_bias = np.asarray(fc_bias, dtype=np.float32)

    # winograd weight taps U[i, o, dy, tap] (f64 transform, fp16 ship)
    U = np.einsum("tk,oidk->iodt", _G, weight.astype(np.float64))
    ut_host = np.ascontiguousarray(
        U.reshape(ICC, 128, OCC, 128, K * NT)
        .transpose(0, 1, 2, 3, 4)
        .reshape(ICC, 128, OCC, 128 * K * NT)
        .astype(np.float16)
    )
    # demod w2[i, o]
    w2_host = np.ascontiguousarray(
        (weight.astype(np.float64) ** 2).sum(axis=(2, 3)).T.astype(np.float32)
    )
    fcw_host = np.ascontiguousarray(fc_weight.T)        # [S, IC]
    fcb_host = np.ascontiguousarray(fc_bias.reshape(IC, 1))

    # x: pad rows to 66, cols to 68; phase de-interleave (c = 4t+k -> [k][t]);
    # fp16; split into two 35-row halves
    xpad = np.zeros((B, IC, H + 2, PW), dtype=np.float32)
    xpad[:, :, 1:H + 1, 1:W + 1] = x
    xr = xpad.reshape(B, ICC, 128, H + 2, NPH, 4).transpose(0, 1, 2, 3, 5, 4)
    halves = np.stack([xr[:, :, :, 0:HR], xr[:, :, :, 31:66]], axis=3)
    xph_host = np.ascontiguousarray(
        halves.transpose(0, 1, 3, 2, 4, 5, 6)
        .reshape(B, ICC, 2, 128, HR * PW)
        .astype(np.float16)
    )

    in_maps = []
    for c in range(NCORES):
        sl = slice(c * BL, (c + 1) * BL)
        in_maps.append({
            "xph": np.ascontiguousarray(xph_host[sl]),
            "ut": ut_host,
            "w2d": w2_host,
            "fcw": fcw_host,
            "st": np.ascontiguousarray(style[sl].T),
            "fcb": fcb_host,
        })
    return in_maps


def kernel(x, style, weight, fc_weight, fc_bias):
    runner = _get_runner()
    in_maps = _prep_inputs(x, style, weight, fc_weight, fc_bias)
    dev_args = runner.put_inputs(in_maps)
    outs = runner.run(dev_args)
    res = runner.results(outs)
    out = np.concatenate([res[c]["y"] for c in range(NCORES)], axis=0)
    return out.astype(np.float32)


# revision 13
# speedup vs baseline: 1.2967x; 1.0873x over previous
"""EqualizedModulatedConv2d (StyleGAN2) Trainium2 kernel.

Strategy: data-parallel over batch B=16 across 8 NeuronCores (2 samples/core).
Conv algorithm: F(4,3) Winograd along the x-dim (6 taps -> 4 output cols),
direct accumulation along y (3 dy taps folded into the PSUM accumulation),
fp16 matmul operands with fp32 PSUM accumulate.

Per core:
  1. style FC (PE, f32) -> es[i,b] = elr*(lin*(style @ fcW.T)[b,i] + fc_bias)
  2. demod norm from host-precomputed w2[i,o]: denom = elr^2 * (w2.T @ es^2),
     norm = rsqrt(denom + 1e-8)   (PE + Act + DVE, tiny)
  3. modulate: xm = x * es (Act engine, fp16), x shipped fp16,
     column-phase-deinterleaved (4 phases x 17) so Winograd input-transform
     reads are stride-1 (enables the DVE 4x perf mode)
  4. input transform: 12 scalar_tensor_tensor ops per (sample, half, icChunk)
     building V[6 taps][35 rows][16 x-tiles] fp16
  5. conv: per (sample, half, rowTile16, ocChunk): 6 taps x 3 dy x 4 ic = 72
     fp16 matmuls, free dim 256 (16 rows x 16 tiles), accumulating 6 tap
     planes in PSUM
  6. inverse transform A^T m: pair-sums on Pool engine, combines on DVE
     (fp16, 4x mode), demod scale + col re-interleave on Act, DMA out.

Host side: winograd weight transform U = G @ w (f64->fp16), w2 = sum(w^2),
x padding + phase deinterleave + fp16 cast, per-core batch sharding.
"""
import numpy as np

B, IC, OC, K, H, W, S = 16, 512, 512, 3, 64, 64, 512
NCORES = 8
BL = B // NCORES          # samples per core
ICC = IC // 128
OCC = OC // 128
SC = S // 128
NT = 6                    # winograd taps F(4,3)
XT = W // 4               # 16 x-tiles per row
NPH = 17                  # phase width (68 padded cols / 4 phases)
PW = 4 * NPH              # 68 padded width
HR = 35                   # rows per half (padded rows 0..34 / 31..65)
RT = 16                   # output rows per conv group
ELR = (2.0 / (IC * K * K)) ** 0.5
LIN = (2.0 / S) ** 0.5

_CACHE = {}

# F(4,3) winograd input transform B^T (host side)
_BT = np.array([
    [4, 0, -5, 0, 1, 0],
    [0, -4, -4, 1, 1, 0],
    [0, 4, -4, -1, 1, 0],
    [0, -2, -1, 2, 1, 0],
    [0, 2, -1, -2, 1, 0],
    [0, 4, 0, -5, 0, 1],
], dtype=np.float64)

# F(4,3) winograd weight transform (host side, f64)
_G = np.array([
    [1 / 4, 0, 0],
    [-1 / 6, -1 / 6, -1 / 6],
    [-1 / 6, 1 / 6, -1 / 6],
    [1 / 24, 1 / 12, 1 / 6],
    [1 / 24, -1 / 12, 1 / 6],
    [0, 0, 1],
], dtype=np.float64)


def _build():
    import concourse.bacc as bacc
    import concourse.mybir as mybir
    import concourse.tile as tile

    f32 = mybir.dt.float32
    f16 = mybir.dt.float16
    ALU = mybir.AluOpType
    AF = mybir.ActivationFunctionType

    nc = bacc.Bacc(None, target_bir_lowering=False, debug=False)
    xph = nc.dram_tensor("xph", [BL, ICC, 2, 128, NT * HR * XT], f16,
                         kind="ExternalInput").ap()
    ut = nc.dram_tensor("ut", [ICC, 128, OCC, 128 * K * NT], f16,
                        kind="ExternalInput").ap()
    w2d = nc.dram_tensor("w2d", [IC, OC], f32, kind="ExternalInput").ap()
    fcw = nc.dram_tensor("fcw", [S, IC], f32, kind="ExternalInput").ap()
    st = nc.dram_tensor("st", [S, BL], f32, kind="ExternalInput").ap()
    fcb = nc.dram_tensor("fcb", [IC, 1], f32, kind="ExternalInput").ap()
    y = nc.dram_tensor("y", [BL, OC, H, W], f32, kind="ExternalOutput").ap()

    with tile.TileContext(nc) as tc:
        with (
            tc.tile_pool(name="up", bufs=1) as up,
            tc.tile_pool(name="fcp", bufs=1) as fcp,
            tc.tile_pool(name="sml", bufs=1) as sml,
            tc.tile_pool(name="xin", bufs=4) as xinp,
            tc.tile_pool(name="vp", bufs=2) as vp,
            tc.tile_pool(name="ivp", bufs=2) as ivp,
            tc.tile_pool(name="outp", bufs=3) as outp,
            tc.tile_pool(name="acc", bufs=2, space="PSUM") as accp,
            tc.tile_pool(name="pacc", bufs=2, space="PSUM") as paccp,
        ):
            # ---- param DMAs (sync queue) ----
            st_sb = fcp.tile([128, SC, BL], f32)
            nc.sync.dma_start(st_sb[:], st.rearrange("(sc p) b -> p sc b", p=128))
            fcb_sb = fcp.tile([128, ICC], f32)
            nc.sync.dma_start(fcb_sb[:], fcb.rearrange("(ic p) z -> p (ic z)", p=128))
            fcw_sb = fcp.tile([128, SC, IC], f32)
            nc.sync.dma_start(fcw_sb[:], fcw.rearrange("(sc p) i -> p sc i", p=128))
            w2_sb = fcp.tile([128, ICC, OC], f32)
            nc.sync.dma_start(w2_sb[:], w2d.rearrange("(ic p) o -> p ic o", p=128))

            # ---- U taps DMA (gpsimd queue, oc-major so oc0 lands first) ----
            u_sb = up.tile([128, ICC, OCC, 128, K, NT], f16)
            for oc in range(OCC):
                for ic in range(ICC):
                    nc.gpsimd.dma_start(
                        u_sb[:, ic, oc].rearrange("p o a b -> p (o a b)"),
                        ut[ic, :, oc, :],
                    )

            # ---- style FC -> es[i, b] = elr*s ----
            ebias = sml.tile([128, ICC], f32)
            nc.scalar.mul(ebias[:], fcb_sb[:], ELR)
            es_sb = sml.tile([128, ICC, BL], f32)
            ss_sb = sml.tile([128, ICC, BL], f32)
            for ic in range(ICC):
                ps = paccp.tile([128, BL], f32, tag="pp")
                for sc in range(SC):
                    nc.tensor.matmul(
                        ps[:], fcw_sb[:, sc, ic * 128:(ic + 1) * 128],
                        st_sb[:, sc, :],
                        start=(sc == 0), stop=(sc == SC - 1),
                    )
                nc.scalar.activation(
                    es_sb[:, ic, :], ps[:], AF.Identity,
                    bias=ebias[:, ic:ic + 1], scale=ELR * LIN,
                )
            nc.vector.tensor_mul(
                ss_sb.rearrange("p i b -> p (i b)"),
                es_sb.rearrange("p i b -> p (i b)"),
                es_sb.rearrange("p i b -> p (i b)"),
            )

            # ---- demod norm: norm[o, oc, b] ----
            norm_sb = sml.tile([128, OCC, BL], f32)
            sqd = sml.tile([128, OCC, BL], f32)
            eps_sb = sml.tile([128, 1], f32)
            nc.vector.memset(eps_sb[:], 1e-8)
            for oc in range(OCC):
                pd = paccp.tile([128, BL], f32, tag="pp")
                for ic in range(ICC):
                    nc.tensor.matmul(
                        pd[:], w2_sb[:, ic, oc * 128:(oc + 1) * 128],
                        ss_sb[:, ic, :],
                        start=(ic == 0), stop=(ic == ICC - 1),
                    )
                nc.scalar.activation(
                    sqd[:, oc, :], pd[:], AF.Sqrt,
                    bias=eps_sb[:],
                )
                nc.vector.reciprocal(norm_sb[:, oc, :], sqd[:, oc, :])

            # ---- V build: DMA host-pretransformed taps, modulate by es
            # (DVE tensor_scalar, 4x perf mode) ----
            def build_v(vtile, b, h, ic):
                xin = xinp.tile([128, NT * HR * XT], f16, tag="xin")
                nc.sync.dma_start(xin[:], xph[b, ic, h])
                vflat = vtile.rearrange("p i t r x -> p i (t r x)")
                nc.vector.tensor_scalar_mul(
                    vflat[:, ic], xin[:], es_sb[:, ic, b:b + 1],
                )

            # ---- conv group: 72 matmuls + inverse + demod + store ----
            def conv_group(vtile, b, h, rt, oc):
                ps = accp.tile([128, NT, RT * XT], f32, tag="acc")
                base = 16 * rt + (1 if h else 0)
                osl = slice(oc * 128, (oc + 1) * 128)
                for tap in range(NT):
                    pview = ps[:, tap, :]
                    for dy in range(K):
                        r0 = base + dy
                        for ic in range(ICC):
                            nc.tensor.matmul(
                                pview,
                                u_sb[:, ic, oc, :, dy, tap],
                                vtile[:, ic, tap, r0:r0 + RT, :].rearrange(
                                    "p r t -> p (r t)"),
                                start=(dy == 0 and ic == 0),
                                stop=(dy == K - 1 and ic == ICC - 1),
                            )
                # inverse transform: Act drains PSUM -> fp16 SBUF, then DVE
                # combines with 2x-mode tensor_tensor / 4x tensor_scalar ops
                c13 = ivp.tile([128, 2, RT * XT], f16, tag="c13")
                nc.scalar.copy(c13[:], ps[:, 1:5:2, :])
                c24 = ivp.tile([128, 2, RT * XT], f16, tag="c24")
                nc.scalar.copy(c24[:], ps[:, 2:6:2, :])
                c05 = ivp.tile([128, 2, RT * XT], f16, tag="c05")
                nc.scalar.copy(c05[:], ps[:, 0:6:5, :])
                PR = ivp.tile([128, 2, RT * XT], f16, tag="PR")
                QS = ivp.tile([128, 2, RT * XT], f16, tag="QS")
                nc.vector.tensor_add(PR[:], c13[:], c24[:])
                nc.vector.tensor_sub(QS[:], c13[:], c24[:])
                sc2 = ivp.tile([128, 2, RT * XT], f16, tag="sc2")
                nc.vector.tensor_add(sc2[:, 0], PR[:, 0], PR[:, 1])
                s8 = ivp.tile([128, 2, RT * XT], f16, tag="s8")
                nc.vector.tensor_scalar_mul(s8[:, 0], QS[:, 1], 8.0)
                nc.vector.tensor_add(sc2[:, 1], s8[:, 0], QS[:, 0])
                o03 = ivp.tile([128, 2, RT * XT], f16, tag="o03")
                nc.vector.tensor_add(o03[:], c05[:], sc2[:])
                o12 = ivp.tile([128, 2, RT * XT], f16, tag="o12")
                nc.vector.tensor_scalar_mul(s8[:, 1], QS[:, 1], 2.0)
                nc.vector.tensor_add(o12[:, 0], s8[:, 1], QS[:, 0])
                nc.vector.tensor_scalar_mul(s8[:, 0], PR[:, 1], 4.0)
                nc.vector.tensor_add(o12[:, 1], s8[:, 0], PR[:, 0])
                # demod scale + column re-interleave on Act
                ot = outp.tile([128, RT * W], f32, tag="ot")
                ov = ot.rearrange("p (r t four) -> p r t four", four=4, t=XT)
                nv = norm_sb[:, oc, b:b + 1]
                o03v = o03.rearrange("p two (r t) -> p two r t", t=XT)
                o12v = o12.rearrange("p two (r t) -> p two r t", t=XT)
                nc.scalar.mul(ov[:, :, :, 0], o03v[:, 0], nv)
                nc.scalar.mul(ov[:, :, :, 1], o12v[:, 0], nv)
                nc.scalar.mul(ov[:, :, :, 2], o12v[:, 1], nv)
                nc.scalar.mul(ov[:, :, :, 3], o03v[:, 1], nv)
                r0g = 32 * h + 16 * rt
                nc.sync.dma_start(
                    y[b, osl, r0g:r0g + RT, :].rearrange("p r c -> p (r c)"),
                    ot[:],
                )

            # ---- main pipeline ----
            steps = [(b, h) for b in range(BL) for h in range(2)]
            vtiles = []
            v0 = vp.tile([128, ICC, NT, HR, XT], f16, tag="V")
            for ic in range(ICC):
                build_v(v0, steps[0][0], steps[0][1], ic)
            vtiles.append(v0)
            for i, (b, h) in enumerate(steps):
                vt = vtiles[i]
                if i + 1 < len(steps):
                    vnext = vp.tile([128, ICC, NT, HR, XT], f16, tag="V")
                    vtiles.append(vnext)
                    nb, nh = steps[i + 1]
                    pending = list(range(ICC))
                else:
                    vnext, pending = None, []
                gi = 0
                for rt in range(2):
                    for oc in range(OCC):
                        conv_group(vt, b, h, rt, oc)
                        if gi < len(pending):
                            build_v(vnext, nb, nh, pending[gi])
                        gi += 1
    nc.compile()
    return nc


class _Runner:
    """Persistent jitted PJRT executor for the SPMD kernel (axon path)."""

    def __init__(self, nc, n_cores):
        import jax
        import numpy as np
        from jax.sharding import Mesh, PartitionSpec
        try:
            from jax.experimental.shard_map import shard_map
        except ImportError:
            from jax.shard_map import shard_map
        import concourse.mybir as mybir
        from concourse.bass2jax import (
            _bass_exec_p, install_neuronx_cc_hook, partition_id_tensor,
        )

        install_neuronx_cc_hook()
        self.jax = jax
        self.n_cores = n_cores
        partition_name = (
            nc.partition_id_tensor.name if nc.partition_id_tensor else None
        )
        in_names, out_names, out_avals, zero_outs = [], [], [], []
        for alloc in nc.m.functions[0].allocations:
            if not isinstance(alloc, mybir.MemoryLocationSet):
                continue
            name = alloc.memorylocations[0].name
            if alloc.kind == "ExternalInput":
                if name != partition_name:
                    in_names.append(name)
            elif alloc.kind == "ExternalOutput":
                out_names.append(name)
                shape = tuple(alloc.tensor_shape)
                dtype = mybir.dt.np(alloc.dtype)
                out_avals.append(jax.core.ShapedArray(shape, dtype))
                zero_outs.append(np.zeros(shape, dtype))
        self.in_names, self.out_names, self.out_avals = in_names, out_names, out_avals

        def _body(*args):
            operands = list(args)
            if partition_name is not None:
                operands.append(partition_id_tensor())
            return tuple(
                _bass_exec_p.bind(
                    *operands,
                    out_avals=tuple(out_avals),
                    in_names=tuple(in_names + out_names + ([partition_name] if partition_name else [])),
                    out_names=tuple(out_names),
                    lowering_input_output_aliases=(),
                    sim_require_finite=False,
                    sim_require_nnan=False,
                    nc=nc,
                )
            )

        devices = jax.devices()[:n_cores]
        mesh = Mesh(np.asarray(devices), ("core",))
        n_params = len(in_names)
        self.fn = jax.jit(
            shard_map(
                _body, mesh=mesh,
                in_specs=(PartitionSpec("core"),) * (n_params + len(out_names)),
                out_specs=(PartitionSpec("core"),) * len(out_names),
                check_rep=False,
            ),
            keep_unused=True,
        )
        self.sharding = jax.sharding.NamedSharding(mesh, PartitionSpec("core"))
        self._dev_zeros = [
            jax.device_put(
                np.zeros((n_cores * z.shape[0], *z.shape[1:]), z.dtype), self.sharding
            )
            for z in zero_outs
        ]

    def put_inputs(self, in_maps):
        concat = [
            np.concatenate(
                [np.asarray(in_maps[c][n]) for c in range(self.n_cores)], axis=0
            )
            for n in self.in_names
        ]
        return [self.jax.device_put(a, self.sharding) for a in concat]

    def run(self, dev_args):
        outs = self.fn(*dev_args, *self._dev_zeros)
        self.jax.block_until_ready(outs)
        return outs

    def results(self, outs):
        res = []
        for c in range(self.n_cores):
            d = {}
            for i, name in enumerate(self.out_names):
                full = np.asarray(outs[i])
                d[name] = full.reshape(self.n_cores, *self.out_avals[i].shape)[c]
            res.append(d)
        return res


def _get_runner():
    if "runner" not in _CACHE:
        nc = _build()
        _CACHE["nc"] = nc
        _CACHE["runner"] = _Runner(nc, NCORES)
    return _CACHE["runner"]


def _prep_inputs(x, style, weight, fc_weight, fc_bias):
    """Host-side sharding + layout marshalling. Returns per-core input maps."""
    x = np.asarray(x, dtype=np.float32)
    style = np.asarray(style, dtype=np.float32)
    weight = np.asarray(weight, dtype=np.float32)
    fc_weight = np.asarray(fc_weight, dtype=np.float32)
    fc_bias = np.asarray(fc_bias, dtype=np.float32)

    # winograd weight taps U[i, o, dy, tap] (f64 transform, fp16 ship)
    U = np.einsum("tk,oidk->iodt", _G, weight.astype(np.float64))
    ut_host = np.ascontiguousarray(
        U.reshape(ICC, 128, OCC, 128, K * NT)
        .transpose(0, 1, 2, 3, 4)
        .reshape(ICC, 128, OCC, 128 * K * NT)
        .astype(np.float16)
    )
    # demod w2[i, o]
    w2_host = np.ascontiguousarray(
        (weight.astype(np.float64) ** 2).sum(axis=(2, 3)).T.astype(np.float32)
    )
    fcw_host = np.ascontiguousarray(fc_weight.T)        # [S, IC]
    fcb_host = np.ascontiguousarray(fc_bias.reshape(IC, 1))

    # x: pad to 66 rows x 68 cols, winograd F(4,3) input transform along x
    # (host, f32), fp16, split into two 35-row halves
    xpad = np.zeros((B, IC, H + 2, PW), dtype=np.float32)
    xpad[:, :, 1:H + 1, 1:W + 1] = x
    cols = 4 * np.arange(XT)
    d = np.stack([xpad[:, :, :, cols + k] for k in range(NT)], axis=2)
    # V[b, i, tap, row, xtile] = sum_k BT[tap, k] * d[b, i, k, row, xtile]
    V = np.einsum("tk,bikrx->bitrx", _BT.astype(np.float32), d)
    Vr = V.reshape(B, ICC, 128, NT, H + 2, XT)
    halves = np.stack([Vr[:, :, :, :, 0:HR], Vr[:, :, :, :, 31:66]], axis=3)
    xph_host = np.ascontiguousarray(
        halves.transpose(0, 1, 3, 2, 4, 5, 6)
        .reshape(B, ICC, 2, 128, NT * HR * XT)
        .astype(np.float16)
    )

    in_maps = []
    for c in range(NCORES):
        sl = slice(c * BL, (c + 1) * BL)
        in_maps.append({
            "xph": np.ascontiguousarray(xph_host[sl]),
            "ut": ut_host,
            "w2d": w2_host,
            "fcw": fcw_host,
            "st": np.ascontiguousarray(style[sl].T),
            "fcb": fcb_host,
        })
    return in_maps


def kernel(x, style, weight, fc_weight, fc_bias):
    runner = _get_runner()
    in_maps = _prep_inputs(x, style, weight, fc_weight, fc_bias)
    dev_args = runner.put_inputs(in_maps)
    outs = runner.run(dev_args)
    res = runner.results(outs)
    out = np.concatenate([res[c]["y"] for c in range(NCORES)], axis=0)
    return out.astype(np.float32)
